# revision 6
# baseline (speedup 1.0000x reference)
"""Trainium2 Bass kernel for nn_CompressedInteractionNetwork_9105330667837.

Algorithm: the network output is (B,1) only, so the 3-layer CIN collapses
algebraically to a per-(b,d)-column quartic form evaluated as
    out[b] = B_const + sum_d [ g(x).t(x) + x.u(x) ],   x = x0[b,:,d] in R^32
with g[o] = x^T W1[o] x (64 quadratic forms), t[k] = x^T U3[k] x + V2[k].x,
u = Asym x + s23.  All quadratic forms are evaluated through a shared
"squares basis": z = LIN @ x (pair-sums), basis = [z^2; x^2; x_m x_{m+16}; x],
then [g;t] = R @ basis.  Everything contracts on the PE in float32r; squares
on ScalarE; products/reduction on VectorE/GpSimd.

Sharding: data-parallel over batch across 8 cores (weights replicated).
"""

import numpy as np
from contextlib import ExitStack

import concourse.bass as bass
from concourse import bacc
import concourse.mybir as mybir
import concourse.tile as tile
from concourse.bass_utils import run_bass_kernel_spmd

B, F, D = 2048, 32, 64
NCORES = 8
BC = B // NCORES            # 256 batches per core
CHUNK_B = 8                 # batches per chunk
P = CHUNK_B * D             # 512 pairs per chunk
NCHUNK = BC // CHUNK_B      # 32

SPECIAL = [(m, m + 16) for m in range(16)]          # pairs done as direct products
_SP = set(SPECIAL)
PAIRS = [(a, b) for a in range(F) for b in range(a + 1, F) if (a, b) not in _SP]
assert len(PAIRS) == 480

f32 = mybir.dt.float32
f32r = mybir.dt.float32r


def fold_weights(W1, b1, W2, b2, W3, b3, W_out, b_out):
    """Host-side folding. Returns dict of small fp32 arrays + bconst float."""
    W1, b1, W2, b2, W3, b3, W_out, b_out = [
        np.asarray(a, dtype=np.float64) for a in (W1, b1, W2, b2, W3, b3, W_out, b_out)
    ]
    w1, w2, w3 = W_out[0:64, 0], W_out[64:128, 0], W_out[128:192, 0]

    V2 = np.einsum("o,ohm->hm", w2, W2)           # (64,32)
    V3 = np.einsum("o,ohm->hm", w3, W3)           # (64,32)
    U3 = np.einsum("hkm,hn->kmn", W2, V3)         # (64,32,32)
    V1 = np.einsum("o,ohm->hm", w1, W1)           # (32,32)
    Le = np.einsum("k,kmn->mn", b1, U3)           # (32,32)
    A = V1 + Le
    Asym = (A + A.T) / 2
    s23 = V2.T @ b1 + V3.T @ b2                   # (32,)
    bconst = D * (w1 @ b1 + w2 @ b2 + w3 @ b3) + b_out[0]

    M1s = (W1 + W1.transpose(0, 2, 1)) / 2        # 64 sym forms for g
    U3s = (U3 + U3.transpose(0, 2, 1)) / 2        # 64 sym forms for t

    # LIN lhsT: (32, 4*128). Tile j rows: j<3 -> PAIRS[128j:128j+128] sums;
    # tile 3 -> PAIRS[384:480] sums (96 rows) + Asym rows (32).
    LINW = np.zeros((F, 4 * 128))
    for j in range(4):
        rows = PAIRS[128 * j: 128 * (j + 1)]
        for i, (a, b_) in enumerate(rows):
            LINW[a, 128 * j + i] += 1.0
            LINW[b_, 128 * j + i] += 1.0
        if j == 3:
            LINW[:, 128 * 3 + 96: 128 * 3 + 128] = Asym.T  # rows 96..127 = Asym @ x

    # Big-matmul lhsT per chain: RW (128, 5*128): RW[k, 128j+? ...] wait layout:
    # lhsT for chain j is (K_j, 128): RW[0:K_j, j-block], K_j = 128 (j<4) or 48.
    # outputs: m<64 -> form M1s[m], v=0 ; m>=64 -> form U3s[m-64], v=V2[m-64]
    forms = np.concatenate([M1s, U3s], axis=0)    # (128, 32, 32)
    linv = np.concatenate([np.zeros((64, F)), V2], axis=0)  # (128, 32)

    # rw layout: rw[k, 128*j + m] = weight of chain-j basis-row k for output m
    RW = np.zeros((128, 5 * 128))
    # chains 0-2: squares of pair-sums
    for j in range(3):
        rows = PAIRS[128 * j: 128 * (j + 1)]
        for i, (a, b_) in enumerate(rows):
            RW[i, 128 * j:128 * (j + 1)] = forms[:, a, b_]
    # chain 3: rows 0-95 squares of PAIRS[384:480]; rows 96-127 x^2
    for i, (a, b_) in enumerate(PAIRS[384:480]):
        RW[i, 128 * 3:128 * 4] = forms[:, a, b_]
    # x^2 weights: S[m,m] - sum_{(a,b) in PAIRS containing m} S[a,b]
    corr = np.zeros((128, F))
    for (a, b_) in PAIRS:
        corr[:, a] += forms[:, a, b_]
        corr[:, b_] += forms[:, a, b_]
    for m in range(F):
        RW[96 + m, 128 * 3:128 * 4] = forms[:, m, m] - corr[:, m]
    # chain 4 (K=48): rows 0-15 direct products x_m x_{m+16}; rows 16-47 x
    for i, (a, b_) in enumerate(SPECIAL):
        RW[i, 128 * 4:128 * 5] = 2.0 * forms[:, a, b_]
    for m in range(F):
        RW[16 + m, 128 * 4:128 * 5] = linv[:, m]

    return {
        "linw": LINW.astype(np.float32),
        "rw": RW.astype(np.float32),
        "s23": s23.reshape(F, 1).astype(np.float32),
        "ones": np.ones((96, 1), dtype=np.float32),
    }, float(bconst)


_module_cache = {}


def build_module(bconst: float):
    key = round(bconst, 12)
    if key in _module_cache:
        return _module_cache[key]
    nc = bacc.Bacc("TRN2", target_bir_lowering=False)
    x_d = nc.dram_tensor("x", [BC, F, D], f32r, kind="ExternalInput")
    linw_d = nc.dram_tensor("linw", [F, 4 * 128], f32r, kind="ExternalInput")
    rw_d = nc.dram_tensor("rw", [128, 5 * 128], f32r, kind="ExternalInput")
    s23_d = nc.dram_tensor("s23", [F, 1], f32, kind="ExternalInput")
    ones_d = nc.dram_tensor("ones", [96, 1], f32, kind="ExternalInput")
    out_d = nc.dram_tensor("out", [1, BC], f32, kind="ExternalOutput")

    SQ = mybir.ActivationFunctionType.Square
    CP = mybir.ActivationFunctionType.Copy
    ADD = mybir.AluOpType.add
    MULT = mybir.AluOpType.mult

    with tile.TileContext(nc) as tc, ExitStack() as ctx:
        const = ctx.enter_context(tc.tile_pool(name="const", bufs=1))
        xp = ctx.enter_context(tc.tile_pool(name="xp", bufs=3))
        chp = ctx.enter_context(tc.tile_pool(name="chp", bufs=10))
        ch4p = ctx.enter_context(tc.tile_pool(name="ch4p", bufs=3))
        prp = ctx.enter_context(tc.tile_pool(name="prp", bufs=2))
        collp = ctx.enter_context(tc.tile_pool(name="collp", bufs=1))
        outp = ctx.enter_context(tc.tile_pool(name="outp", bufs=1))
        linps = ctx.enter_context(tc.tile_pool(name="linps", bufs=5, space="PSUM"))
        bigps = ctx.enter_context(tc.tile_pool(name="bigps", bufs=2, space="PSUM"))
        finps = ctx.enter_context(tc.tile_pool(name="finps", bufs=1, space="PSUM"))

        linw_t = const.tile([F, 4 * 128], f32r)
        nc.sync.dma_start(linw_t[:], linw_d[:])
        rw_t = const.tile([128, 5 * 128], f32r)
        nc.sync.dma_start(rw_t[:], rw_d[:])
        s23_t = const.tile([F, 1], f32)
        nc.sync.dma_start(s23_t[:], s23_d[:])
        ones_t = const.tile([96, 1], f32)
        nc.sync.dma_start(ones_t[:], ones_d[:])

        coll = collp.tile([96, BC], f32)

        for c in range(NCHUNK):
            xsrc = x_d[c * CHUNK_B:(c + 1) * CHUNK_B].transpose([1, 0, 2])
            x_t = xp.tile([F, P], f32r, tag="x")
            nc.sync.dma_start(
                x_t[:].rearrange("k (b d) -> k b d", b=CHUNK_B), xsrc
            )
            ch4 = ch4p.tile([48, P], f32r, tag="ch4")
            nc.sync.dma_start(
                ch4[16:48].rearrange("k (b d) -> k b d", b=CHUNK_B), xsrc
            )
            xs_t = xp.tile([16, P], f32r, tag="xs")
            nc.sync.dma_start(
                xs_t[:].rearrange("k (b d) -> k b d", b=CHUNK_B),
                x_d[c * CHUNK_B:(c + 1) * CHUNK_B, 16:32, :].transpose([1, 0, 2]),
            )

            lps = []
            for j in range(4):
                lp = linps.tile([128, P], f32, tag="lp")
                nc.tensor.matmul(
                    lp[:],
                    linw_t[:, 128 * j:128 * (j + 1)],
                    x_t[:],
                    start=True, stop=True,
                )
                lps.append(lp)

            chains = []
            for j in range(3):
                chj = chp.tile([128, P], f32r, tag="ch")
                nc.scalar.activation(chj[:], lps[j][:], SQ)
                chains.append(chj)
            ch3 = chp.tile([128, P], f32r, tag="ch")
            nc.scalar.activation(ch3[0:96], lps[3][0:96], SQ)
            nc.gpsimd.tensor_mul(ch3[96:128], x_t[:], x_t[:])
            chains.append(ch3)
            nc.vector.tensor_mul(ch4[0:16], x_t[0:16], xs_t[:])
            chains.append(ch4)

            bp = bigps.tile([128, P], f32, tag="bp")
            for j in range(5):
                K_j = 128 if j < 4 else 48
                nc.tensor.matmul(
                    bp[:],
                    rw_t[0:K_j, 128 * j:128 * (j + 1)],
                    chains[j][0:K_j],
                    start=(j == 0), stop=(j == 4),
                )

            pr = prp.tile([96, P], f32, tag="pr")
            gs = prp.tile([64, P], f32, tag="gs")
            nc.scalar.activation(gs[:], bp[0:64], CP)
            nc.vector.tensor_mul(pr[0:64], gs[:], bp[64:128])
            # x .* (u + s23) ; u rows live in lin psum tile 3 rows 96..127
            nc.vector.scalar_tensor_tensor(
                pr[64:96], lps[3][96:128], s23_t[:], x_t[:], ADD, MULT
            )
            # reduce over d (innermost 64) -> (96, 8) into collector slice
            nc.vector.tensor_reduce(
                coll[:, c * CHUNK_B:(c + 1) * CHUNK_B],
                pr[:].rearrange("p (b d) -> p b d", b=CHUNK_B),
                axis=mybir.AxisListType.X,
                op=ADD,
            )

        fp = finps.tile([1, BC], f32)
        nc.tensor.matmul(
            fp[:], ones_t[:], coll[:],
            start=True, stop=True,
        )
        out_sb = outp.tile([1, BC], f32)
        nc.scalar.activation(out_sb[:], fp[:], CP, bias=float(bconst))
        nc.sync.dma_start(out_d[:], out_sb[:])

    nc.compile()
    _module_cache[key] = nc
    return nc


def _run(inputs, trace=False, **kw):
    folded, bconst = fold_weights(
        inputs["W1"], inputs["b1"], inputs["W2"], inputs["b2"],
        inputs["W3"], inputs["b3"], inputs["W_out"], inputs["b_out"],
    )
    nc = build_module(bconst)
    x0 = np.ascontiguousarray(np.asarray(inputs["x0"], dtype=np.float32))
    in_maps = []
    for c in range(NCORES):
        m = dict(folded)
        m["x"] = np.ascontiguousarray(x0[BC * c:BC * (c + 1)])
        in_maps.append(m)
    res = run_bass_kernel_spmd(nc, in_maps, core_ids=list(range(NCORES)),
                               trace=trace, **kw)
    out = np.concatenate(
        [res.results[c]["out"].reshape(BC, 1) for c in range(NCORES)], axis=0
    )
    return out, res


def kernel(**inputs) -> np.ndarray:
    out, _ = _run(inputs, trace=False)
    return out


# revision 9
# speedup vs baseline: 518.9112x; 518.9112x over previous
"""Trainium2 Bass kernel for nn_CompressedInteractionNetwork_9105330667837.

Algorithm: the network output is (B,1) only, so the 3-layer CIN collapses
algebraically to a per-(b,d)-column quartic form evaluated as
    out[b] = B_const + sum_d [ g(x).t(x) + x.u(x) ],   x = x0[b,:,d] in R^32
with g[o] = x^T W1[o] x (64 quadratic forms), t[k] = x^T U3[k] x + V2[k].x,
u = Asym x + s23.  All quadratic forms are evaluated through a shared
"squares basis": z = LIN @ x (pair-sums), basis = [z^2; x^2; x_m x_{m+16}; x],
then [g;t] = R @ basis.  Everything contracts on the PE in float32r; squares
on ScalarE; products/reduction on VectorE/GpSimd.

Sharding: data-parallel over batch across 8 cores (weights replicated).
"""

import numpy as np
from contextlib import ExitStack

import concourse.bass as bass
from concourse import bacc
import concourse.mybir as mybir
import concourse.tile as tile
from concourse.bass_utils import run_bass_kernel_spmd

B, F, D = 2048, 32, 64
NCORES = 8
BC = B // NCORES            # 256 batches per core
CHUNK_B = 8                 # batches per chunk
P = CHUNK_B * D             # 512 pairs per chunk
NCHUNK = BC // CHUNK_B      # 32

SPECIAL = [(m, m + 16) for m in range(16)]          # pairs done as direct products
_SP = set(SPECIAL)
PAIRS = [(a, b) for a in range(F) for b in range(a + 1, F) if (a, b) not in _SP]
assert len(PAIRS) == 480

f32 = mybir.dt.float32
f32r = mybir.dt.float32r


def fold_weights(W1, b1, W2, b2, W3, b3, W_out, b_out):
    """Host-side folding. Returns dict of small fp32 arrays + bconst float."""
    W1, b1, W2, b2, W3, b3, W_out, b_out = [
        np.asarray(a, dtype=np.float64) for a in (W1, b1, W2, b2, W3, b3, W_out, b_out)
    ]
    w1, w2, w3 = W_out[0:64, 0], W_out[64:128, 0], W_out[128:192, 0]

    V2 = np.einsum("o,ohm->hm", w2, W2)           # (64,32)
    V3 = np.einsum("o,ohm->hm", w3, W3)           # (64,32)
    U3 = np.einsum("hkm,hn->kmn", W2, V3)         # (64,32,32)
    V1 = np.einsum("o,ohm->hm", w1, W1)           # (32,32)
    Le = np.einsum("k,kmn->mn", b1, U3)           # (32,32)
    A = V1 + Le
    Asym = (A + A.T) / 2
    s23 = V2.T @ b1 + V3.T @ b2                   # (32,)
    bconst = D * (w1 @ b1 + w2 @ b2 + w3 @ b3) + b_out[0]

    M1s = (W1 + W1.transpose(0, 2, 1)) / 2        # 64 sym forms for g
    U3s = (U3 + U3.transpose(0, 2, 1)) / 2        # 64 sym forms for t

    # LIN lhsT: (32, 4*128). Tile j rows: j<3 -> PAIRS[128j:128j+128] sums;
    # tile 3 -> PAIRS[384:480] sums (96 rows) + Asym rows (32).
    LINW = np.zeros((F, 4 * 128))
    for j in range(4):
        rows = PAIRS[128 * j: 128 * (j + 1)]
        for i, (a, b_) in enumerate(rows):
            LINW[a, 128 * j + i] += 1.0
            LINW[b_, 128 * j + i] += 1.0
        if j == 3:
            LINW[:, 128 * 3 + 96: 128 * 3 + 128] = Asym.T  # rows 96..127 = Asym @ x

    # Big-matmul lhsT per chain: RW (128, 5*128): RW[k, 128j+? ...] wait layout:
    # lhsT for chain j is (K_j, 128): RW[0:K_j, j-block], K_j = 128 (j<4) or 48.
    # outputs: m<64 -> form M1s[m], v=0 ; m>=64 -> form U3s[m-64], v=V2[m-64]
    forms = np.concatenate([M1s, U3s], axis=0)    # (128, 32, 32)
    linv = np.concatenate([np.zeros((64, F)), V2], axis=0)  # (128, 32)

    # rw layout: rw[k, 128*j + m] = weight of chain-j basis-row k for output m
    RW = np.zeros((128, 5 * 128))
    # chains 0-2: squares of pair-sums
    for j in range(3):
        rows = PAIRS[128 * j: 128 * (j + 1)]
        for i, (a, b_) in enumerate(rows):
            RW[i, 128 * j:128 * (j + 1)] = forms[:, a, b_]
    # chain 3: rows 0-95 squares of PAIRS[384:480]; rows 96-127 x^2
    for i, (a, b_) in enumerate(PAIRS[384:480]):
        RW[i, 128 * 3:128 * 4] = forms[:, a, b_]
    # x^2 weights: S[m,m] - sum_{(a,b) in PAIRS containing m} S[a,b]
    corr = np.zeros((128, F))
    for (a, b_) in PAIRS:
        corr[:, a] += forms[:, a, b_]
        corr[:, b_] += forms[:, a, b_]
    for m in range(F):
        RW[96 + m, 128 * 3:128 * 4] = forms[:, m, m] - corr[:, m]
    # chain 4 (K=48): rows 0-15 direct products x_m x_{m+16}; rows 16-47 x
    for i, (a, b_) in enumerate(SPECIAL):
        RW[i, 128 * 4:128 * 5] = 2.0 * forms[:, a, b_]
    for m in range(F):
        RW[16 + m, 128 * 4:128 * 5] = linv[:, m]

    return {
        "linw": LINW.astype(np.float32),
        "rw": RW.astype(np.float32),
        "s23": s23.reshape(F, 1).astype(np.float32),
        "ones": np.ones((96, 1), dtype=np.float32),
    }, float(bconst)


_module_cache = {}


def build_module(bconst: float, repeat: int = 1):
    key = (round(bconst, 12), repeat)
    if key in _module_cache:
        return _module_cache[key]
    nc = bacc.Bacc("TRN2", target_bir_lowering=False)
    x_d = nc.dram_tensor("x", [BC, F, D], f32r, kind="ExternalInput")
    linw_d = nc.dram_tensor("linw", [F, 4 * 128], f32r, kind="ExternalInput")
    rw_d = nc.dram_tensor("rw", [128, 5 * 128], f32r, kind="ExternalInput")
    s23_d = nc.dram_tensor("s23", [F, 1], f32, kind="ExternalInput")
    ones_d = nc.dram_tensor("ones", [96, 1], f32, kind="ExternalInput")
    out_d = nc.dram_tensor("out", [1, BC], f32, kind="ExternalOutput")

    SQ = mybir.ActivationFunctionType.Square
    CP = mybir.ActivationFunctionType.Copy
    ADD = mybir.AluOpType.add
    MULT = mybir.AluOpType.mult

    with tile.TileContext(nc) as tc, ExitStack() as ctx:
        const = ctx.enter_context(tc.tile_pool(name="const", bufs=1))
        xp = ctx.enter_context(tc.tile_pool(name="xp", bufs=3))
        chp = ctx.enter_context(tc.tile_pool(name="chp", bufs=10))
        ch4p = ctx.enter_context(tc.tile_pool(name="ch4p", bufs=3))
        prp = ctx.enter_context(tc.tile_pool(name="prp", bufs=2))
        collp = ctx.enter_context(tc.tile_pool(name="collp", bufs=1))
        outp = ctx.enter_context(tc.tile_pool(name="outp", bufs=1))
        linps = ctx.enter_context(tc.tile_pool(name="linps", bufs=5, space="PSUM"))
        bigps = ctx.enter_context(tc.tile_pool(name="bigps", bufs=2, space="PSUM"))
        finps = ctx.enter_context(tc.tile_pool(name="finps", bufs=1, space="PSUM"))

        linw_t = const.tile([F, 4 * 128], f32r)
        nc.sync.dma_start(linw_t[:], linw_d[:])
        rw_t = const.tile([128, 5 * 128], f32r)
        nc.sync.dma_start(rw_t[:], rw_d[:])
        s23_t = const.tile([F, 1], f32)
        nc.sync.dma_start(s23_t[:], s23_d[:])
        ones_t = const.tile([96, 1], f32)
        nc.sync.dma_start(ones_t[:], ones_d[:])

        coll = collp.tile([96, BC], f32)

        rep_ctx = tc.For_i(0, repeat, 1) if repeat > 1 else None
        if rep_ctx is not None:
            ctx.enter_context(rep_ctx)

        GROUP = 4
        for g in range(NCHUNK // GROUP):
            b0 = g * GROUP * CHUNK_B
            nb = GROUP * CHUNK_B
            xsrc = x_d[b0:b0 + nb].transpose([1, 0, 2])   # (32, nb, 64)
            xg_t = xp.tile([F, GROUP * P], f32r, tag="x")
            nc.sync.dma_start(
                xg_t[:].rearrange("k (b d) -> k b d", b=nb), xsrc
            )
            ch4g = ch4p.tile([48, GROUP * P], f32r, tag="ch4")
            nc.sync.dma_start(
                ch4g[16:48].rearrange("k (b d) -> k b d", b=nb), xsrc
            )
            xs_g = xp.tile([16, GROUP * P], f32r, tag="xs")
            nc.sync.dma_start(
                xs_g[:].rearrange("k (b d) -> k b d", b=nb),
                x_d[b0:b0 + nb, 16:32, :].transpose([1, 0, 2]),
            )
            for ci in range(GROUP):
                cs = slice(ci * P, (ci + 1) * P)
                x_t = xg_t[:, cs]
                ch4 = ch4g[:, cs]

                lps = []
                for j in range(4):
                    lp = linps.tile([128, P], f32, tag="lp")
                    nc.tensor.matmul(
                        lp[:],
                        linw_t[:, 128 * j:128 * (j + 1)],
                        x_t,
                        start=True, stop=True,
                    )
                    lps.append(lp)

                chains = []
                for j in range(3):
                    chj = chp.tile([128, P], f32r, tag="ch")
                    nc.scalar.activation(chj[:], lps[j][:], SQ)
                    chains.append(chj[:])
                ch3 = chp.tile([128, P], f32r, tag="ch")
                nc.scalar.activation(ch3[0:96], lps[3][0:96], SQ)
                nc.gpsimd.tensor_mul(ch3[96:128], x_t, x_t)
                chains.append(ch3[:])
                nc.vector.tensor_mul(ch4[0:16], x_t[0:16], xs_g[:, cs])
                chains.append(ch4)

                bp = bigps.tile([128, P], f32, tag="bp")
                for j in range(5):
                    K_j = 128 if j < 4 else 48
                    nc.tensor.matmul(
                        bp[:],
                        rw_t[0:K_j, 128 * j:128 * (j + 1)],
                        chains[j][0:K_j],
                        start=(j == 0), stop=(j == 4),
                    )

                pr = prp.tile([96, P], f32, tag="pr")
                gs = prp.tile([64, P], f32, tag="gs")
                nc.scalar.activation(gs[:], bp[0:64], CP)
                nc.vector.tensor_mul(pr[0:64], gs[:], bp[64:128])
                nc.vector.scalar_tensor_tensor(
                    pr[64:96], lps[3][96:128], s23_t[:], x_t, ADD, MULT
                )
                c = g * GROUP + ci
                nc.vector.tensor_reduce(
                    coll[:, c * CHUNK_B:(c + 1) * CHUNK_B],
                    pr[:].rearrange("p (b d) -> p b d", b=CHUNK_B),
                    axis=mybir.AxisListType.X,
                    op=ADD,
                )

        fp = finps.tile([1, BC], f32)
        nc.tensor.matmul(
            fp[:], ones_t[:], coll[:],
            start=True, stop=True,
        )
        out_sb = outp.tile([1, BC], f32)
        nc.scalar.activation(out_sb[:], fp[:], CP, bias=float(bconst))
        nc.sync.dma_start(out_d[:], out_sb[:])

    nc.compile()
    _module_cache[key] = nc
    return nc


def _run(inputs, trace=False, **kw):
    folded, bconst = fold_weights(
        inputs["W1"], inputs["b1"], inputs["W2"], inputs["b2"],
        inputs["W3"], inputs["b3"], inputs["W_out"], inputs["b_out"],
    )
    nc = build_module(bconst)
    x0 = np.ascontiguousarray(np.asarray(inputs["x0"], dtype=np.float32))
    in_maps = []
    for c in range(NCORES):
        m = dict(folded)
        m["x"] = np.ascontiguousarray(x0[BC * c:BC * (c + 1)])
        in_maps.append(m)
    res = run_bass_kernel_spmd(nc, in_maps, core_ids=list(range(NCORES)),
                               trace=trace, **kw)
    out = np.concatenate(
        [res.results[c]["out"].reshape(BC, 1) for c in range(NCORES)], axis=0
    )
    return out, res


def kernel(**inputs) -> np.ndarray:
    out, _ = _run(inputs, trace=False)
    return out


# revision 24
# speedup vs baseline: 667.0320x; 1.2854x over previous
"""Trainium2 Bass kernel for nn_CompressedInteractionNetwork_9105330667837.

Algorithm: the network output is (B,1) only, so the 3-layer CIN collapses
algebraically to a per-(b,d)-column quartic form evaluated as
    out[b] = B_const + sum_d [ g(x).t(x) + x.u(x) ],   x = x0[b,:,d] in R^32
with g[o] = x^T W1[o] x (64 quadratic forms), t[k] = x^T U3[k] x + V2[k].x,
u = Asym x + s23.  All quadratic forms are evaluated through a shared
"squares basis": z = LIN @ x (pair-sums), basis = [z^2; x^2; x_m x_{m+16}; x],
then [g;t] = R @ basis.  Everything contracts on the PE in float32r; squares
on ScalarE; products/reduction on VectorE/GpSimd.

Sharding: data-parallel over batch across 8 cores (weights replicated).
"""

import numpy as np
from contextlib import ExitStack

import concourse.bass as bass
from concourse import bacc
import concourse.mybir as mybir
import concourse.tile as tile
from concourse.bass_utils import run_bass_kernel_spmd
from concourse import dve_ops as _dvo
from concourse.dve_spec import Spec as _Spec, Src0 as _Src0, Bin as _Bin, AluOp as _AluOp
from concourse.dve_table_gen import dve_ver_for as _dve_ver_for


def _register_square_op():
    if "SQUARE_ANT" in _dvo._SUB_OPCODE_FOR_NAME:
        return _dvo.CUSTOM_DVE_SPECS and [op for op in _dvo.OPS if op.name == "SQUARE_ANT"][0]
    op = _dvo.DveOp(
        "SQUARE_ANT",
        _Spec(
            body=_Bin(_AluOp.MULTIPLY, _Src0, _Src0),
            reference=lambda in0, in1, s0, s1, imm2: (
                in0.astype(np.float32) * in0.astype(np.float32)
            ),
        ),
        subdim=False,
        uops_sha={},
    )
    _dvo.OPS.append(op)
    _dvo.CUSTOM_DVE_SPECS[op.name] = op.spec
    _dvo._SUB_OPCODE_FOR_NAME[op.name] = max(_dvo._SUB_OPCODE_FOR_NAME.values()) + 1
    for ver in ("v3", "v4"):
        try:
            op.compile(ver)
        except ValueError as e:
            import re as _re
            m = _re.search(r": ([0-9a-f]{16}) ", str(e))
            if m is None:
                raise
            op.uops_sha[ver] = m.group(1)
            _dvo._COMPILE_CACHE.pop((op.name, ver), None)
            op.compile(ver)
    return op


SQUARE_ANT = _register_square_op()

B, F, D = 2048, 32, 64
NCORES = 8
BC = B // NCORES            # 256 batches per core
CHUNK_B = 8                 # batches per chunk
P = CHUNK_B * D             # 512 pairs per chunk
NCHUNK = BC // CHUNK_B      # 32

SPECIAL = [(m, m + 16) for m in range(16)]          # pairs done as direct products
_SP = set(SPECIAL)
PAIRS = [(a, b) for a in range(F) for b in range(a + 1, F) if (a, b) not in _SP]
assert len(PAIRS) == 480

f32 = mybir.dt.float32
f32r = mybir.dt.float32r


def fold_weights(W1, b1, W2, b2, W3, b3, W_out, b_out):
    """Host-side folding. Returns dict of small fp32 arrays + bconst float."""
    W1, b1, W2, b2, W3, b3, W_out, b_out = [
        np.asarray(a, dtype=np.float64) for a in (W1, b1, W2, b2, W3, b3, W_out, b_out)
    ]
    w1, w2, w3 = W_out[0:64, 0], W_out[64:128, 0], W_out[128:192, 0]

    V2 = np.einsum("o,ohm->hm", w2, W2)           # (64,32)
    V3 = np.einsum("o,ohm->hm", w3, W3)           # (64,32)
    U3 = np.einsum("hkm,hn->kmn", W2, V3)         # (64,32,32)
    V1 = np.einsum("o,ohm->hm", w1, W1)           # (32,32)
    Le = np.einsum("k,kmn->mn", b1, U3)           # (32,32)
    A = V1 + Le
    Asym = (A + A.T) / 2
    s23 = V2.T @ b1 + V3.T @ b2                   # (32,)
    bconst = D * (w1 @ b1 + w2 @ b2 + w3 @ b3) + b_out[0]

    M1s = (W1 + W1.transpose(0, 2, 1)) / 2        # 64 sym forms for g
    U3s = (U3 + U3.transpose(0, 2, 1)) / 2        # 64 sym forms for t

    # LIN lhsT: (32, 4*128). Tile j rows: j<3 -> PAIRS[128j:128j+128] sums;
    # tile 3 -> PAIRS[384:480] sums (96 rows) + Asym rows (32).
    LINW = np.zeros((F, 4 * 128))
    for j in range(4):
        rows = PAIRS[128 * j: 128 * (j + 1)]
        for i, (a, b_) in enumerate(rows):
            LINW[a, 128 * j + i] += 1.0
            LINW[b_, 128 * j + i] += 1.0
        if j == 3:
            LINW[:, 128 * 3 + 96: 128 * 3 + 128] = Asym.T  # rows 96..127 = Asym @ x

    # Big-matmul lhsT per chain: RW (128, 5*128): RW[k, 128j+? ...] wait layout:
    # lhsT for chain j is (K_j, 128): RW[0:K_j, j-block], K_j = 128 (j<4) or 48.
    # outputs: m<64 -> form M1s[m], v=0 ; m>=64 -> form U3s[m-64], v=V2[m-64]
    forms = np.concatenate([M1s, U3s], axis=0)    # (128, 32, 32)
    linv = np.concatenate([np.zeros((64, F)), V2], axis=0)  # (128, 32)

    # rw layout: rw[k, 128*j + m] = weight of chain-j basis-row k for output m
    RW = np.zeros((128, 5 * 128))
    # chains 0-2: squares of pair-sums
    for j in range(3):
        rows = PAIRS[128 * j: 128 * (j + 1)]
        for i, (a, b_) in enumerate(rows):
            RW[i, 128 * j:128 * (j + 1)] = forms[:, a, b_]
    # chain 3: rows 0-95 squares of PAIRS[384:480]; rows 96-127 x^2
    for i, (a, b_) in enumerate(PAIRS[384:480]):
        RW[i, 128 * 3:128 * 4] = forms[:, a, b_]
    # x^2 weights: S[m,m] - sum_{(a,b) in PAIRS containing m} S[a,b]
    corr = np.zeros((128, F))
    for (a, b_) in PAIRS:
        corr[:, a] += forms[:, a, b_]
        corr[:, b_] += forms[:, a, b_]
    # chain 4 (K=80): rows 0-31 x^2; 32-63 x; 64-79 products x_m x_{m+16}
    for i, (a, b_) in enumerate(SPECIAL):
        RW[64 + i, 128 * 4:128 * 5] = 2.0 * forms[:, a, b_]
    for m in range(F):
        RW[32 + m, 128 * 4:128 * 5] = linv[:, m]
        RW[m, 128 * 4:128 * 5] = forms[:, m, m] - corr[:, m]

    return {
        "linw": LINW.astype(np.float32),
        "rw": RW.astype(np.float32),
        "s23": s23.reshape(F, 1).astype(np.float32),
        "ones": np.ones((96, 1), dtype=np.float32),
    }, float(bconst)


_module_cache = {}


CFG = {"P": 512, "lin_split": 2, "lin_bufs": 2, "big_bufs": 1, "gp_d16": True,
       "dve_sq": True,
       "no_sq": False, "no_gs": False, "no_dve": False, "no_gp": False,
       "no_lin": False, "no_big": False, "no_dma": False}


def build_module(bconst: float, repeat: int = 1):
    key = (round(bconst, 12), repeat, tuple(sorted(CFG.items())))
    if key in _module_cache:
        return _module_cache[key]
    nc = bacc.Bacc("TRN2", target_bir_lowering=False)
    x_d = nc.dram_tensor("x", [BC, F, D], f32r, kind="ExternalInput")
    linw_d = nc.dram_tensor("linw", [F, 4 * 128], f32r, kind="ExternalInput")
    rw_d = nc.dram_tensor("rw", [128, 5 * 128], f32r, kind="ExternalInput")
    s23_d = nc.dram_tensor("s23", [F, 1], f32, kind="ExternalInput")
    ones_d = nc.dram_tensor("ones", [96, 1], f32r, kind="ExternalInput")
    out_d = nc.dram_tensor("out", [1, BC], f32, kind="ExternalOutput")

    SQ = mybir.ActivationFunctionType.Square
    CP = mybir.ActivationFunctionType.Copy
    ADD = mybir.AluOpType.add
    MULT = mybir.AluOpType.mult

    with tile.TileContext(nc) as tc, ExitStack() as ctx:
        const = ctx.enter_context(tc.tile_pool(name="const", bufs=1))
        xp = ctx.enter_context(tc.tile_pool(name="xp", bufs=3))
        chp = ctx.enter_context(tc.tile_pool(name="chp", bufs=10))
        ch4p = ctx.enter_context(tc.tile_pool(name="ch4p", bufs=3))
        prp = ctx.enter_context(tc.tile_pool(name="prp", bufs=1))
        gsp = ctx.enter_context(tc.tile_pool(name="gsp", bufs=3))
        outp = ctx.enter_context(tc.tile_pool(name="outp", bufs=1))
        linps = ctx.enter_context(
            tc.tile_pool(name="linps", bufs=CFG["lin_bufs"], space="PSUM"))
        bigps = ctx.enter_context(tc.tile_pool(name="bigps", bufs=CFG["big_bufs"], space="PSUM"))
        finps = ctx.enter_context(tc.tile_pool(name="finps", bufs=1, space="PSUM"))

        linw_t = const.tile([F, 4 * 128], f32r)
        nc.sync.dma_start(linw_t[:], linw_d[:])
        rw_t = const.tile([128, 5 * 128], f32r)
        nc.sync.dma_start(rw_t[:], rw_d[:])
        s23_t = const.tile([F, 1], f32)
        nc.sync.dma_start(s23_t[:], s23_d[:])
        ones_t = const.tile([96, 1], f32r)
        nc.sync.dma_start(ones_t[:], ones_d[:])

        pr_all = prp.tile([96, BC * D], f32r)

        rep_ctx = tc.For_i(0, repeat, 1) if repeat > 1 else None
        if rep_ctx is not None:
            ctx.enter_context(rep_ctx)

        CP_ = CFG["P"]          # pairs per chunk
        CB = CP_ // D           # batches per chunk
        NCH = BC // CB          # chunks
        NT = CP_ // 512         # matmul N-tiles per chunk
        GROUP = max(1, 2048 // CP_)
        for g in range(NCH // GROUP):
            b0 = g * GROUP * CB
            nb = GROUP * CB
            xsrc = x_d[b0:b0 + nb].transpose([1, 0, 2])   # (32, nb, 64)
            xg_t = xp.tile([F, GROUP * CP_], f32r, tag="x")
            nc.sync.dma_start(
                xg_t[:].rearrange("k (b d) -> k b d", b=nb), xsrc
            )
            ch4g = ch4p.tile([80, GROUP * CP_], f32r, tag="ch4")
            nc.sync.dma_start(
                ch4g[32:64].rearrange("k (b d) -> k b d", b=nb), xsrc
            )
            xs_g = xp.tile([16, GROUP * CP_], f32r, tag="xs")
            nc.sync.dma_start(
                xs_g[:].rearrange("k (b d) -> k b d", b=nb),
                x_d[b0:b0 + nb, 16:32, :].transpose([1, 0, 2]),
            )
            for ci in range(GROUP):
                cs = slice(ci * CP_, (ci + 1) * CP_)
                x_t = xg_t[:, cs]
                ch4 = ch4g[:, cs]

                ns = CFG["lin_split"]  # chains per lin psum tile
                ntile = 4 // ns
                lptiles = []
                chains = []
                for t in range(ntile):
                    lp = linps.tile([128, ns * CP_], f32, tag="lp")
                    for jj in range(ns):
                        j = t * ns + jj
                        for nt in range(NT):
                            nc.tensor.matmul(
                                lp[:, jj * CP_ + nt * 512:jj * CP_ + (nt + 1) * 512],
                                linw_t[:, 128 * j:128 * (j + 1)],
                                x_t[:, nt * 512:(nt + 1) * 512],
                                start=True, stop=True,
                            )
                    lptiles.append(lp)
                    chn = chp.tile([128, ns * CP_], f32r, tag="ch")
                    c_ = g * GROUP + ci
                    if t == ntile - 1 and ntile > 1 and c_ % 2 == 1 and CFG["dve_sq"]:
                        nc.vector._custom_dve(SQUARE_ANT, out=chn[:], in0=lp[:])
                    else:
                        nc.scalar.activation(chn[:], lp[:], SQ)
                    for jj in range(ns):
                        chains.append(chn[:, jj * CP_:(jj + 1) * CP_])
                lp_u = lptiles[-1]
                nc.gpsimd.tensor_mul(ch4[0:32], x_t, x_t)
                if CFG["gp_d16"]:
                    nc.gpsimd.tensor_mul(ch4[64:80], x_t[0:16], xs_g[:, cs])
                else:
                    nc.vector.tensor_mul(ch4[64:80], x_t[0:16], xs_g[:, cs])
                chains.append(ch4)

                bp = bigps.tile([128, CP_], f32, tag="bp")
                for j in range(5):
                    K_j = 128 if j < 4 else 80
                    for nt in range(NT):
                        nc.tensor.matmul(
                            bp[:, nt * 512:(nt + 1) * 512],
                            rw_t[0:K_j, 128 * j:128 * (j + 1)],
                            chains[j][0:K_j, nt * 512:(nt + 1) * 512],
                            start=(j == 0), stop=(j == 4),
                        )

                c = g * GROUP + ci
                pcs = slice(c * CP_, (c + 1) * CP_)
                gs = gsp.tile([64, CP_], f32, tag="gs")
                nc.scalar.activation(gs[:], bp[0:64], CP)
                nc.vector.tensor_mul(pr_all[0:64, pcs], gs[:], bp[64:128])
                nc.vector.scalar_tensor_tensor(
                    pr_all[64:96, pcs], lp_u[96:128, (ns - 1) * CP_:ns * CP_],
                    s23_t[:], x_t, ADD, MULT
                )

        fp = finps.tile([1, BC], f32)
        pr3 = pr_all[:].rearrange("p (b d) -> p b d", b=BC)
        for d in range(D):
            nc.tensor.matmul(
                fp[:], ones_t[:], pr3[:, :, d],
                start=(d == 0), stop=(d == D - 1),
            )
        out_sb = outp.tile([1, BC], f32)
        nc.scalar.activation(out_sb[:], fp[:], CP, bias=float(bconst))
        nc.sync.dma_start(out_d[:], out_sb[:])

    nc.compile()
    _module_cache[key] = nc
    return nc


def _run(inputs, trace=False, **kw):
    folded, bconst = fold_weights(
        inputs["W1"], inputs["b1"], inputs["W2"], inputs["b2"],
        inputs["W3"], inputs["b3"], inputs["W_out"], inputs["b_out"],
    )
    nc = build_module(bconst)
    x0 = np.ascontiguousarray(np.asarray(inputs["x0"], dtype=np.float32))
    in_maps = []
    for c in range(NCORES):
        m = dict(folded)
        m["x"] = np.ascontiguousarray(x0[BC * c:BC * (c + 1)])
        in_maps.append(m)
    res = run_bass_kernel_spmd(nc, in_maps, core_ids=list(range(NCORES)),
                               trace=trace, **kw)
    out = np.concatenate(
        [res.results[c]["out"].reshape(BC, 1) for c in range(NCORES)], axis=0
    )
    return out, res


def kernel(**inputs) -> np.ndarray:
    out, _ = _run(inputs, trace=False)
    return out
